# revision 10
# baseline (speedup 1.0000x reference)
"""Trainium2 Bass kernel for a 2-layer GCN (GCNConv x2 + mean-pool + FC), v2.

Distribution: nodes sharded across 8 cores (edge partitioning by dst owner),
weights replicated. Per-layer the 64-wide scaled node table is AllGathered in
4 slices (pipelined with the edge sweep); each core's edge sweep gathers
per-edge source rows with dma_gather and scatters them into per-dst-block
PSUM accumulators via one-hot matmuls.

v2 over v1:
  - slice-major global table: AllGather split into 4 per-slice collectives
    that overlap the sweep (bucket s of the gather only needs slice s).
  - whole-layer idx stream resident in SBUF (no per-call idx DMA loads).
  - deep (bufs=8) tile pools so descgen never stalls on gather-DMA drain
    latency; private-DRAM bounce of the AllGather output before gathering.
  - L1 post-processing and L2 tail folded into the sweeps at block
    completion (scalar-engine ops); AllGather-2 slices fire mid-sweep-1;
    +b1 pre-added as sqrt(deg)*b1 during phase B (dinv*sqrt(deg)==1).

Math: GCN symmetric normalization factorizes: coef = dinv[s]*dinv[d], so
    agg = dinv * (segsum(dinv[s] * xw[s]) + dinv[d]*xw[d])
and (A_hat @ h) @ W == A_hat @ (h @ W), so both layers aggregate 64-wide
rows (H1 = 64); W2 is applied after the second aggregation.
"""

import sys

import numpy as np

if "/opt/trn_rl_repo" not in sys.path:
    sys.path.insert(0, "/opt/trn_rl_repo")

from concourse import bacc, bass, mybir, tile
from concourse.bass_utils import run_bass_kernel_spmd
from concourse.masks import make_identity

FP32 = mybir.dt.float32
BF16 = mybir.dt.bfloat16
I16 = mybir.dt.int16
I32 = mybir.dt.int32

P = 128
NCORES = 8
NSLICE = 4


def _ceil(a, b):
    return (a + b - 1) // b


# --------------------------------------------------------------------------
# Host-side prep: shard nodes, partition/sort/pad edges, pack device layouts.
# --------------------------------------------------------------------------
def _prep(x, edge_src, edge_dst, W1, b1, W2, b2, Wfc, bfc, cap=1024):
    N, F = x.shape
    H1 = W1.shape[1]
    H2 = W2.shape[1]
    C = Wfc.shape[1]
    PN = _ceil(N, NCORES)          # nodes per core (logical)
    NB = _ceil(PN, P)              # dst blocks per core
    PNP = NB * P                   # padded nodes per core
    TBL = NCORES * PNP             # global (padded) table rows

    # 4 slices over local blocks -> slice-major global table
    base, rem = NB // NSLICE, NB % NSLICE
    sb = np.array([base + 1] * rem + [base] * (NSLICE - rem))  # blocks/slice
    # front-load a small slice 0 so AllGather slice 0 (and its private
    # bounce) completes early and the edge sweep starts sooner
    shift = int(sb[0]) // 2
    if shift and NSLICE > 1:
        add, r = divmod(shift, NSLICE - 1)
        sb[0] -= shift
        sb[1:] += add
        sb[1 : 1 + r] += 1
    sl_rows = sb * P                                  # local rows per slice
    C_s = np.concatenate([[0], np.cumsum(sl_rows)])   # local row offset
    Bs = NCORES * C_s                                 # table row offset
    blk2s = np.repeat(np.arange(NSLICE), sb)          # local block -> slice
    assert C_s[-1] == PNP and Bs[-1] == TBL
    assert sl_rows.max() * NCORES <= 32768            # int16 idx per bucket

    src = np.asarray(edge_src).astype(np.int64)
    dst = np.asarray(edge_dst).astype(np.int64)
    c_arr = src // PN
    loc = src % PN
    s_arr = blk2s[loc >> 7]
    idxb = c_arr * sl_rows[s_arr] + (loc - C_s[s_arr])  # idx within bucket
    dst_core = dst // PN
    dst_loc = dst % PN

    # pass 1: per-(core, bucket, block) counts -> shared padded group sizes
    per_core = []
    cnt = np.zeros((NCORES, NSLICE, NB), np.int64)
    for c in range(NCORES):
        m = dst_core == c
        per_core.append((idxb[m], dst_loc[m], s_arr[m]))
        for s in range(NSLICE):
            cnt[c, s] = np.bincount(per_core[c][1][per_core[c][2] == s] >> 7,
                                    minlength=NB)

    Gbk = ((cnt.max(axis=0) + P - 1) // P) * P   # [NSLICE, NB]
    TOT = int(Gbk.sum())
    NW = TOT // P

    # group offsets (bucket-major, block-inner)
    goff = np.zeros((NSLICE, NB), np.int64)
    off = 0
    for s in range(NSLICE):
        for blk in range(NB):
            goff[s, blk] = off
            off += int(Gbk[s, blk])
    assert off == TOT

    # fold bucket per block: last bucket with a group for blk
    fold_bucket = np.zeros(NB, np.int64)
    for blk in range(NB):
        nz = np.nonzero(Gbk[:, blk])[0]
        assert len(nz) > 0
        fold_bucket[blk] = nz[-1]

    # calls: pack whole groups (split oversize) into <= cap idxs, per bucket.
    # calls[i] = (bucket, off, n); segs[i] = [(w0, nwin, blk, gfirst, glast)]
    calls = []
    segs = []
    for s in range(NSLICE):
        cur_off, cur_n, cur_segs = None, 0, []
        for blk in range(NB):
            g = int(Gbk[s, blk])
            if g == 0:
                continue
            gpos = 0
            while gpos < g:
                take = min(g - gpos, cap - cur_n if cur_n else cap, cap)
                if cur_n == 0:
                    cur_off = int(goff[s, blk]) + gpos
                cur_segs.append((cur_n // P, take // P, blk,
                                 gpos == 0, gpos + take == g))
                cur_n += take
                gpos += take
                if cur_n == cap or (cur_n % P) != 0:
                    calls.append((s, cur_off, cur_n))
                    segs.append(cur_segs)
                    cur_off, cur_n, cur_segs = None, 0, []
        if cur_n:
            calls.append((s, cur_off, cur_n))
            segs.append(cur_segs)
    assert sum(n for _, _, n in calls) == TOT
    GWMAX = max(n for _, _, n in calls) // P

    # pass 2: fill per-core streams (pads: idx -1, dstoff -1)
    in_maps = []
    for c in range(NCORES):
        idx16 = np.full(TOT, -1, np.int16)
        dstoff = np.full(TOT, -1, np.int16)
        es, ed, sbkt = per_core[c]
        for s in range(NSLICE):
            mb = sbkt == s
            eb = ed[mb]
            eidx = es[mb]
            blk = eb >> 7
            order = np.lexsort((eidx, blk))  # by block, then ascending src
            eb = eb[order]
            eidx = eidx[order]
            blk = blk[order]
            cnts = np.bincount(blk, minlength=NB)
            first = np.concatenate([[0], np.cumsum(cnts)])[:-1]
            rank = np.arange(len(eb)) - first[blk]
            pos = goff[s][blk] + rank
            idx16[pos] = eidx.astype(np.int16)
            dstoff[pos] = (eb & 127).astype(np.int16)

        # pad slots gather row 0 (safe, masked by dstoff=-1 in the S build)
        idx16[idx16 == -1] = 0

        # device layouts
        idx_wrap = idx16.reshape(TOT // 16, 16).T            # [16, TOT/16]
        idx_dram = np.tile(idx_wrap, (8, 1)).copy()          # [128, TOT/16]
        dstoff_dram = dstoff.reshape(NW, P).T.copy()         # [128, NW]

        # rowptr (CSR offsets of dst-sorted local edges) for device-side deg
        ed_all = np.sort(dst_loc[dst_core == c])
        rowptr = np.searchsorted(ed_all, np.arange(PNP + 1)).astype(np.int32)
        rp_lo = rowptr[:PNP].reshape(NB, P).T.copy()         # [128, NB]
        rp_hi = rowptr[1 : PNP + 1].reshape(NB, P).T.copy()  # [128, NB]
        # sqrt(deg) flat [1, PNP] (local node order) for the rank-1 +b1 matmul
        rdflat = np.sqrt(
            (rowptr[1 : PNP + 1] - rowptr[:PNP] + 1).astype(np.float32)
        ).reshape(1, PNP)

        # node features, transposed + padded
        lo, hi = c * PN, min((c + 1) * PN, N)
        xc = np.zeros((PNP, F), np.float32)
        xc[: hi - lo] = np.asarray(x[lo:hi], np.float32)
        xT = xc.T.copy()                                      # [F, PNP]

        KC = F // P
        W1r = (
            np.asarray(W1, np.float32)
            .reshape(KC, P, H1)
            .transpose(1, 0, 2)
            .reshape(P, KC * H1)
            .copy()
        )
        CP = _ceil(C, P)
        Wfc_pad = np.zeros((H2, CP * P), np.float32)
        Wfc_pad[:, :C] = np.asarray(Wfc, np.float32)
        bfc_pad = np.zeros(CP * P, np.float32)
        bfc_pad[:C] = np.asarray(bfc, np.float32)
        bfc_r = bfc_pad.reshape(CP, P).T.copy()               # [128, CP]

        in_maps.append(
            {
                "xT": xT,
                "W1r": W1r,
                "b1": np.tile(np.asarray(b1, np.float32).reshape(1, H1), (P, 1)),
                "W2": np.asarray(W2, np.float32),
                "b2": np.asarray(b2, np.float32).reshape(H2, 1),
                "Wfc": Wfc_pad,
                "bfcr": bfc_r,
                "rp_lo": rp_lo,
                "rp_hi": rp_hi,
                "rdflat": rdflat,
                "idx": idx_dram,
                "dstoff": dstoff_dram,
            }
        )

    cfg = dict(
        N=N, F=F, H1=H1, H2=H2, C=C, PN=PN, NB=NB, PNP=PNP, TBL=TBL,
        TOT=TOT, NW=NW, calls=calls, segs=segs, cap=cap, GWMAX=GWMAX,
        CP=_ceil(C, P), KC=F // P,
        sl_rows=[int(v) for v in sl_rows], C_s=[int(v) for v in C_s],
        sb=[int(v) for v in sb], fold_bucket=fold_bucket,
        blk2s=[int(v) for v in blk2s],
    )
    return cfg, in_maps


# --------------------------------------------------------------------------
# Device kernel builder
# --------------------------------------------------------------------------
def _build(tc, out_ap, ins, cfg):
    nc = tc.nc
    H1, H2, NB, PNP = cfg["H1"], cfg["H2"], cfg["NB"], cfg["PNP"]
    NW, KC, CP, TOT = cfg["NW"], cfg["KC"], cfg["CP"], cfg["TOT"]
    calls, segs, GWMAX = cfg["calls"], cfg["segs"], cfg["GWMAX"]
    sl_rows, C_s, sb = cfg["sl_rows"], cfg["C_s"], cfg["sb"]
    fold_bucket = cfg["fold_bucket"]
    # last local block of each slice (for AG issuance)
    slice_last_blk = np.cumsum(sb) - 1

    # persistent DRAM buffers
    y1b = nc.dram_tensor("y1b", [PNP, H1], FP32).ap()
    y2b = nc.dram_tensor("y2b", [PNP, H1], FP32).ap()
    Yt1 = [
        nc.dram_tensor(f"Yt1_{s}", [NCORES * sl_rows[s], H1], FP32,
                       addr_space="Shared").ap() if sl_rows[s] else None
        for s in range(NSLICE)
    ]
    Yt2 = [
        nc.dram_tensor(f"Yt2_{s}", [NCORES * sl_rows[s], H1], FP32,
                       addr_space="Shared").ap() if sl_rows[s] else None
        for s in range(NSLICE)
    ]
    if PRIVATE_TABLE:
        Ytp1 = [
            nc.dram_tensor(f"Ytp1_{s}", [NCORES * sl_rows[s], H1], FP32).ap()
            if sl_rows[s] else None
            for s in range(NSLICE)
        ]
        Ytp2 = [
            nc.dram_tensor(f"Ytp2_{s}", [NCORES * sl_rows[s], H1], FP32).ap()
            if sl_rows[s] else None
            for s in range(NSLICE)
        ]
    gsum_d = nc.dram_tensor("gsum_d", [H2, 1], FP32).ap()
    gsum_sh = nc.dram_tensor("gsum_sh", [H2, 1], FP32, addr_space="Shared").ap()

    rg = [list(range(NCORES))]

    def ag(ins_ap, outs_ap, priv_ap=None):
        nc.gpsimd.collective_compute(
            "AllGather", mybir.AluOpType.bypass, ins=[ins_ap], outs=[outs_ap],
            replica_groups=rg,
        )
        if priv_ap is not None:
            nc.sync.dma_start(priv_ap, outs_ap)

    with (
        tc.tile_pool(name="cons", bufs=1) as cons,
        tc.tile_pool(name="aggp", bufs=1) as aggp,
    ):
        # ---- constants + whole-layer idx stream to SBUF ----
        W1s = cons.tile([P, KC * H1], FP32)
        b1s = cons.tile([P, H1], FP32)
        W2s = cons.tile([H1, H2], FP32)
        b2s = cons.tile([H2, 1], FP32)
        Wfcs = cons.tile([H2, CP * P], FP32)
        bfcs = cons.tile([P, CP], FP32)
        dstoff_s = cons.tile([P, NW], I16)
        allidx = cons.tile([P, TOT // 16], I16)
        nc.sync.dma_start(allidx[:], ins["idx"])
        nc.sync.dma_start(W1s[:], ins["W1r"])
        nc.sync.dma_start(b1s[:], ins["b1"])
        nc.sync.dma_start(W2s[:], ins["W2"])
        nc.sync.dma_start(b2s[:], ins["b2"])
        nc.sync.dma_start(Wfcs[:], ins["Wfc"])
        nc.sync.dma_start(bfcs[:], ins["bfcr"])
        nc.sync.dma_start(dstoff_s[:], ins["dstoff"])

        iota_i = cons.tile([P, P], I16)
        nc.gpsimd.iota(iota_i[:], pattern=[[1, P]], base=0, channel_multiplier=0)
        iota_b = cons.tile([P, P], BF16)
        nc.vector.tensor_copy(iota_b[:], iota_i[:])
        dstoff_b = cons.tile([P, NW], BF16)
        nc.vector.tensor_copy(dstoff_b[:], dstoff_s[:])
        ident = cons.tile([P, P], FP32)
        make_identity(nc, ident[:])

        # ---- degree -> dinv ----
        rp_lo = cons.tile([P, NB], I32)
        rp_hi = cons.tile([P, NB], I32)
        nc.sync.dma_start(rp_lo[:], ins["rp_lo"])
        nc.sync.dma_start(rp_hi[:], ins["rp_hi"])
        deg = cons.tile([P, NB], FP32)
        lo_f = cons.tile([P, NB], FP32)
        rec = cons.tile([P, NB], FP32)
        dinv = cons.tile([P, NB], FP32)
        nc.vector.tensor_copy(deg[:], rp_hi[:])
        nc.vector.tensor_copy(lo_f[:], rp_lo[:])
        nc.vector.tensor_sub(deg[:], deg[:], lo_f[:])
        nc.vector.tensor_scalar_add(deg[:], deg[:], 1.0)
        nc.vector.reciprocal(rec[:], deg[:])
        nc.scalar.sqrt(dinv[:], rec[:])
        # rd = sqrt(deg) (dinv*sqrt(deg) == 1, so adding sqrt(deg)*b1 to the
        # aggregate in phase B yields +b1 after the dinv post-scale)
        rd = cons.tile([P, NB], FP32)
        if SCALAR_POST:
            nc.vector.tensor_tensor(out=rd[:], in0=deg[:], in1=dinv[:],
                                    op=mybir.AluOpType.mult)

        # node-major aggregate [128, NB*H1]
        agg = aggp.tile([P, NB * H1], FP32)

        # ---- phase B: xw = x @ W1, y1 = dinv * xw -> agg init + y1 bounce,
        #      issuing AG1 slice collectives as their blocks complete ----
        si = 0
        with (
            tc.tile_pool(name="xload", bufs=4) as xload,
            tc.tile_pool(name="psB", bufs=2, space="PSUM") as psB,
        ):
            for b in range(NB):
                ps = psB.tile([P, H1], FP32)
                for k in range(KC):
                    xt = xload.tile([P, P], FP32)
                    eng = nc.sync if (b * KC + k) % 2 == 0 else nc.scalar
                    eng.dma_start(
                        xt[:], ins["xT"][k * P : (k + 1) * P, b * P : (b + 1) * P]
                    )
                    nc.tensor.matmul(
                        out=ps[:],
                        lhsT=xt[:],
                        rhs=W1s[:, k * H1 : (k + 1) * H1],
                        start=(k == 0),
                        stop=(k == KC - 1),
                    )
                nc.scalar.activation(
                    agg[:, b * H1 : (b + 1) * H1],
                    ps[:],
                    mybir.ActivationFunctionType.Copy,
                    scale=dinv[:, b : b + 1],
                )
                nc.sync.dma_start(
                    y1b[b * P : (b + 1) * P, :], agg[:, b * H1 : (b + 1) * H1]
                )
                if SCALAR_POST:
                    # after the clean y1 bounce, add sqrt(deg)*b1 into the
                    # local aggregate so the L1 fold reduces to relu(dinv*agg)
                    tb1 = xload.tile([P, H1], FP32, tag="tb1")
                    nc.vector.tensor_scalar(tb1[:], b1s[:], rd[:, b : b + 1],
                                            None, mybir.AluOpType.mult)
                    nc.vector.tensor_tensor(
                        out=agg[:, b * H1 : (b + 1) * H1],
                        in0=agg[:, b * H1 : (b + 1) * H1],
                        in1=tb1[:], op=mybir.AluOpType.add)
                while si < NSLICE and b == slice_last_blk[si]:
                    if sl_rows[si] and SLICED_AG:
                        ag(y1b[C_s[si] : C_s[si + 1], :], Yt1[si],
                           Ytp1[si] if PRIVATE_TABLE else None)
                    si += 1
        if not SLICED_AG:
            for s in range(NSLICE):
                if sl_rows[s]:
                    ag(y1b[C_s[s] : C_s[s + 1], :], Yt1[s],
                       Ytp1[s] if PRIVATE_TABLE else None)

        wait_ag2 = [False]

        def post_l1(blk, sl, l1p, psTp, psHp):
            u = l1p.tile([P, H1], FP32, tag="u1")
            if SCALAR_POST:
                # +b1 was already folded into the PSUM accumulator
                nc.scalar.activation(u[:], sl,
                                     mybir.ActivationFunctionType.Relu,
                                     scale=dinv[:, blk : blk + 1])
                nc.scalar.activation(sl, u[:],
                                     mybir.ActivationFunctionType.Copy,
                                     scale=dinv[:, blk : blk + 1])
            else:
                nc.vector.tensor_scalar(u[:], sl, dinv[:, blk : blk + 1], None,
                                        mybir.AluOpType.mult)
                nc.vector.tensor_tensor(out=u[:], in0=u[:], in1=b1s[:],
                                        op=mybir.AluOpType.add)
                nc.vector.tensor_scalar_max(u[:], u[:], 0.0)
                nc.vector.tensor_scalar(sl, u[:], dinv[:, blk : blk + 1], None,
                                        mybir.AluOpType.mult)
            nc.sync.dma_start(y2b[blk * P : (blk + 1) * P, :], sl)
            if SLICED_AG:
                s2 = cfg["blk2s"][blk]
                if blk == slice_last_blk[s2]:
                    ag(y2b[C_s[s2] : C_s[s2 + 1], :], Yt2[s2],
                       Ytp2[s2] if PRIVATE_TABLE else None)

        parts = cons.tile([H2, NB], FP32)

        def post_l2(blk, sl, l2t, psTp, psHp):
            u = l2t.tile([P, H1], FP32, tag="u2")
            if SCALAR_POST:
                nc.scalar.activation(u[:], sl,
                                     mybir.ActivationFunctionType.Copy,
                                     scale=dinv[:, blk : blk + 1])
            else:
                nc.vector.tensor_scalar(u[:], sl, dinv[:, blk : blk + 1], None,
                                        mybir.AluOpType.mult)
            pT = psTp.tile([H1, P], FP32, tag="pT")
            nc.tensor.transpose(out=pT[:], in_=u[:], identity=ident[:])
            uT = l2t.tile([H1, P], FP32, tag="uT")
            if SCALAR_POST:
                nc.scalar.activation(uT[:], pT[:],
                                     mybir.ActivationFunctionType.Copy)
            else:
                nc.vector.tensor_copy(uT[:], pT[:])
            pH = psHp.tile([H2, P], FP32, tag="pH")
            nc.tensor.matmul(out=pH[:], lhsT=W2s[:], rhs=uT[:], start=True,
                             stop=True)
            h2 = l2t.tile([H2, P], FP32, tag="h2")
            nc.scalar.activation(
                h2[:], pH[:], mybir.ActivationFunctionType.Relu, bias=b2s[:, 0:1]
            )
            nv = min(P, cfg["PN"] - blk * P)  # exclude padded nodes
            nc.vector.tensor_reduce(
                parts[:, blk : blk + 1], h2[:, :nv], mybir.AxisListType.X,
                mybir.AluOpType.add,
            )

        def edge_sweep(Yt, post, add_bias=False):
            with (
                tc.tile_pool(name="gbuf", bufs=BUFS) as gbuf,
                tc.tile_pool(name="gbf", bufs=BUFS) as gbfp,
                tc.tile_pool(name="sall", bufs=BUFS) as sallp,
                tc.tile_pool(name="post", bufs=4) as postp,
                tc.tile_pool(name="psW", bufs=4, space="PSUM") as psW,
                tc.tile_pool(name="psT", bufs=2, space="PSUM") as psTp,
                tc.tile_pool(name="psH", bufs=2, space="PSUM") as psHp,
            ):
                ps = None
                for ci, (bkt, off, n) in enumerate(calls):
                    nwin = n // P
                    gt = gbuf.tile([P, GWMAX, H1], FP32, tag="gbuf")
                    nc.gpsimd.dma_gather(
                        out_ap=gt[:, :nwin, :],
                        in_ap=Yt[bkt],
                        idxs_ap=allidx[:, off // 16 : (off + n) // 16],
                        num_idxs=n,
                        num_idxs_reg=n,
                        elem_size=H1,
                        queue_num=ci % 4,
                    )
                    gb = gbfp.tile([P, GWMAX, H1], BF16, tag="gbf")
                    nc.scalar.activation(
                        gb[:, :nwin, :], gt[:, :nwin, :],
                        mybir.ActivationFunctionType.Copy,
                    )
                    wbase = off // P
                    S = sallp.tile([P, GWMAX, P], BF16, tag="sall")
                    nc.vector.tensor_tensor(
                        out=S[:, :nwin, :],
                        in0=dstoff_b[:, wbase : wbase + nwin]
                        .unsqueeze(2)
                        .broadcast_to([P, nwin, P]),
                        in1=iota_b[:].unsqueeze(1).broadcast_to([P, nwin, P]),
                        op=mybir.AluOpType.is_equal,
                    )
                    for (w0, nw, blk, gfirst, glast) in segs[ci]:
                        for w in range(w0, w0 + nw):
                            if gfirst and w == w0:
                                ps = psW.tile([P, H1], FP32, tag="psW")
                            nc.tensor.matmul(
                                out=ps[:],
                                lhsT=S[:, w, :],
                                rhs=gb[:, w, :],
                                start=bool(gfirst and w == w0),
                                stop=bool(glast and w == w0 + nw - 1),
                            )
                        if glast:
                            sl = agg[:, blk * H1 : (blk + 1) * H1]
                            nc.vector.tensor_tensor(
                                out=sl, in0=sl, in1=ps[:], op=mybir.AluOpType.add
                            )
                            if FOLD_POST and bkt == fold_bucket[blk]:
                                post(blk, sl, postp, psTp, psHp)
                if not FOLD_POST:
                    for blk in range(NB):
                        post(blk, agg[:, blk * H1 : (blk + 1) * H1],
                             postp, psTp, psHp)

        # ---- layer-1 sweep (incl. folded post + AG2 slice issuance) ----
        edge_sweep(Ytp1 if PRIVATE_TABLE else Yt1, post_l1, add_bias=True)
        if not SLICED_AG:
            for s in range(NSLICE):
                if sl_rows[s]:
                    ag(y2b[C_s[s] : C_s[s + 1], :], Yt2[s],
                       Ytp2[s] if PRIVATE_TABLE else None)

        # ---- layer-2 sweep (incl. folded tail) ----
        edge_sweep(Ytp2 if PRIVATE_TABLE else Yt2, post_l2)

        # ---- readout: sum -> allreduce -> mean -> FC ----
        gsum = cons.tile([H2, 1], FP32)
        nc.vector.tensor_reduce(gsum[:], parts[:], mybir.AxisListType.X,
                                mybir.AluOpType.add)
        nc.sync.dma_start(gsum_d, gsum[:])
        nc.gpsimd.collective_compute(
            "AllReduce", mybir.AluOpType.add, ins=[gsum_d], outs=[gsum_sh],
            replica_groups=rg,
        )
        gme = cons.tile([H2, 1], FP32)
        nc.sync.dma_start(gme[:], gsum_sh)
        nc.vector.tensor_scalar(gme[:], gme[:], 1.0 / cfg["N"], None,
                                mybir.AluOpType.mult)

        ocol = cons.tile([P, CP], FP32)
        with tc.tile_pool(name="psF", bufs=2, space="PSUM") as psF:
            for cchunk in range(CP):
                pf = psF.tile([P, 1], FP32, tag="pf")
                nc.tensor.matmul(
                    out=pf[:],
                    lhsT=Wfcs[:, cchunk * P : (cchunk + 1) * P],
                    rhs=gme[:],
                    start=True,
                    stop=True,
                )
                nc.vector.tensor_tensor(
                    out=ocol[:, cchunk : cchunk + 1],
                    in0=pf[:],
                    in1=bfcs[:, cchunk : cchunk + 1],
                    op=mybir.AluOpType.add,
                )
        nc.sync.dma_start(out_ap, ocol[:])


# --------------------------------------------------------------------------
# Entry point
# --------------------------------------------------------------------------
_CACHE = {}
TRACE = False
LAST_RESULT = None
CAP = 1024
SCRATCH = 65536
FOLD_POST = True
SLICED_AG = True
BUFS = 8
PRIVATE_TABLE = True
SCALAR_POST = True


def _get_compiled(cfg, in_maps):
    key = (cfg["N"], cfg["F"], cfg["TOT"], cfg["NW"], cfg["cap"], SCRATCH,
           FOLD_POST, SLICED_AG, BUFS, PRIVATE_TABLE, SCALAR_POST)
    if key in _CACHE:
        return _CACHE[key]
    nc = bacc.Bacc(
        "TRN2", target_bir_lowering=False, debug=False, num_devices=NCORES,
        num_swdge_queues=4, dynamic_dma_scratch_size=SCRATCH,
    )
    ins = {
        k: nc.dram_tensor(k, list(v.shape), mybir.dt.from_np(v.dtype),
                          kind="ExternalInput").ap()
        for k, v in in_maps[0].items()
    }
    out_ap = nc.dram_tensor("out", [P, cfg["CP"]], FP32, kind="ExternalOutput").ap()
    with tile.TileContext(nc) as tc:
        _build(tc, out_ap, ins, cfg)
    nc.compile()
    _CACHE[key] = nc
    return nc


def kernel(**inputs):
    global LAST_RESULT
    cfg, in_maps = _prep(**inputs, cap=CAP)
    nc = _get_compiled(cfg, in_maps)
    res = run_bass_kernel_spmd(nc, in_maps, list(range(NCORES)), trace=TRACE)
    LAST_RESULT = res
    o = res.results[0]["out"]  # [128, CP]
    return o.T.reshape(-1)[: cfg["C"]].astype(np.float32)


if __name__ == "__main__":
    # smoke test with tiny synthetic inputs
    rng = np.random.default_rng(0)
    N, E, F, H1, H2, C = 2048, 8192, 256, 64, 32, 1000
    x = rng.standard_normal((N, F), dtype=np.float32)
    es = rng.integers(0, N, E).astype(np.int32)
    ed = rng.integers(0, N, E).astype(np.int32)
    W1 = rng.standard_normal((F, H1), dtype=np.float32) * 0.06
    b1 = np.zeros(H1, np.float32)
    W2 = rng.standard_normal((H1, H2), dtype=np.float32) * 0.12
    b2 = np.zeros(H2, np.float32)
    Wfc = rng.standard_normal((H2, C), dtype=np.float32) * 0.17
    bfc = rng.standard_normal(C).astype(np.float32) * 0.17
    out = kernel(x=x, edge_src=es, edge_dst=ed, W1=W1, b1=b1, W2=W2, b2=b2,
                 Wfc=Wfc, bfc=bfc)
    print(out.shape, out[:8])
